# revision 5
# baseline (speedup 1.0000x reference)
"""NT-Xent contrastive loss (SimCLR) on 8 Trainium2 NeuronCores.

Strategy (v3: host-norm + symmetry + fp8 DoubleRow, PE-only dep chains):
  - Host: z = concat(z_i, z_j) [8192, 1024], L2-normalize rows in f32,
    scale by S=16, quantize to fp8 e4m3. The cosine-sim matrix is then
    just G = q @ q.T (scaled by S^2), no on-device normalization.
  - Symmetry: sim is symmetric. In rotated coords (each core's 1024 rows
    at block 0), core c computes only column blocks 0..4 (5/8 of the
    matrix). Blocks 1-3 contribute BOTH row-side exp-sums (via ACT accum)
    and column-side exp-sums (ones-matmul partition reduction of the exp
    tiles, accumulated over m in PSUM); block 4 row-side only (its
    transpose is block 4 of the peer core); block 0 row-side with the
    self-diagonal masked. Every ordered (r,c) pair of the full 8192x8192
    matrix is covered exactly once across the fleet.
  - PE runs fp8e4 DoubleRow matmuls (2 k-subtiles of 128 per pass,
    0.5 cycles/row = 4x bf16 MAC throughput). Contraction 1024 = 4
    DoubleRow groups accumulated in PSUM.
  - Self-diag masked ON PE: an extra [128,128] fp8 matmul diag(-28) x
    diag(+28) = -784*I appended to the accumulation group (G_diag ~ +256,
    so masked logit ~ -529*INVT2 ~ -29.5 -> exp ~ 1.5e-13, negligible).
    This keeps every ACT exp dependent only on the PE semaphore (no DVE
    writes to PSUM), so Tile emits a single merged sem wait per exp and
    the multi-wait splitter leaves ACT alone.
  - Column-side reduction also on PE (ones-matmul over partitions,
    PSUM-accumulated across m), emitted one m-step late so PE never
    stalls waiting for ACT.
  - Host combines: per-row sumexp = own row-side + 3 column-side chunks
    from neighbor cores; loss = mean(log(sumexp) - pos/(S^2*T)).

This container's walrus build only accepts ONE semaphore wait per
instruction (and none on CTRL-encoded ones like Drain), while Tile freely
emits several. Two workarounds below: the TileContext epilogue drain's waits
are re-emitted on DVE memsets, and a post-pass splits any multi-wait
instruction by inserting single-wait no-op "carrier" clones (per-engine
templates) just before it on the same engine stream. The PE carrier is a
[128,1] LDWEIGHTS (harmless: every real matmul loads its own weights).
"""

import copy

import numpy as np
import ml_dtypes


def _install_tile_drain_patch():
    import concourse.tile as tile
    from concourse import mybir
    from concourse.vector_clock import ScopedClock

    if getattr(tile.TileContext, "_drain_patch_installed", False):
        return

    def _drain_and_barrier(self, tick_clock, wait_clock):
        nc = self.nc
        drain_inst = nc.sync.drain()
        wait_clock.add_sem_waits(
            drain_inst.ins, ScopedClock({None: tick_clock.global_clock})
        )
        waits = list(drain_inst.ins.sync_info.on_wait)
        drain_inst.ins.sync_info.on_wait.clear()

        if waits:
            scr = nc.const_aps.tensor(0.0, (1, 1), mybir.dt.float32)
            for w in waits:
                ms = nc.vector.memset(scr, 0)
                if ms.ins.sync_info is None:
                    ms.ins.sync_info = mybir.SyncInfo(on_wait=[], on_update=[])
                ms.ins.sync_info.on_wait.append(w)

        nc.all_engine_barrier()
        assert self.sems is not None
        popped = nc._tile_sem_poison_stack.pop()
        assert popped is self._sem_poison
        nc.clear_and_free_semaphores(list(self.sems.allocated().values()))
        nc.all_engine_barrier()

    tile.TileContext._drain_and_barrier = _drain_and_barrier
    tile.TileContext._drain_patch_installed = True


_install_tile_drain_patch()

import concourse.bass as bass
import concourse.tile as tile
from concourse import mybir
from concourse.bass_utils import run_bass_kernel_spmd
from concourse.masks import make_identity

P = 128
D = 1024
R = 8192          # 2N rows
MY = 1024         # rows per core (= block size)
NB = 5            # column blocks computed per core (symmetry: 0..4)
KT = 8            # 128-deep k-subtiles in D
DKT = 4           # DoubleRow groups (256-deep each)
MT = 8            # m-tiles per core
CW = 1024         # column chunk width (= one block)
TEMP = 0.07
S = 16.0          # fp8 pre-scale; G = S^2 * sim
DMV = 28.0        # diag-mask matmul operand: adds -DMV^2 to self-sim
INVT2 = float(1.0 / (S * S * TEMP))
FP8 = mybir.dt.float8e4
BF16 = mybir.dt.bfloat16
F32 = mybir.dt.float32
ALU = mybir.AluOpType
ACTF = mybir.ActivationFunctionType
DR = mybir.MatmulPerfMode.DoubleRow

TRACE = False          # set True externally (test harness) for NTFF profiling
LAST_RESULTS = None    # BassKernelResults of the last run (for the harness)

_NC_CACHE = None


def _split_multi_waits(nc, templates):
    """Rewrite any instruction carrying >1 sem waits: keep the last wait,
    move each extra onto a fresh single-wait clone of the same-engine no-op
    template inserted immediately before it (engine streams are in-order)."""
    n = 0
    for f in nc.m.functions:
        for bb in f.blocks:
            newlist = []
            for ins in bb.instructions:
                si = getattr(ins, "sync_info", None)
                if si is not None and si.on_wait and len(si.on_wait) > 1:
                    extras = list(si.on_wait[:-1])
                    keep = list(si.on_wait[-1:])
                    tmpl = templates.get(ins.engine)
                    assert tmpl is not None, (
                        f"no wait-carrier template for engine {ins.engine} "
                        f"({type(ins).__name__} {ins.name})"
                    )
                    for w in extras:
                        c = copy.deepcopy(tmpl)
                        c.name = f"wcarrier-{n}"
                        n += 1
                        c.sync_info = mybir.SyncInfo(on_wait=[w], on_update=[])
                        newlist.append(c)
                    del si.on_wait[:]
                    si.on_wait.extend(keep)
                newlist.append(ins)
            bb.instructions[:] = newlist
    return n


def build():
    nc = bass.Bass()
    # [jc][p][ks][col]: element (p, ks, col) of chunk jc = q_rot[jc*CW+col,
    # ks*128+p]; flattened to rows jc*128+p, free ks*CW+col.
    zt = nc.dram_tensor("zt", [NB * P, KT * CW], FP8, kind="ExternalInput")
    # [:, :128] = diag(-DMV), [:, 128:] = diag(+DMV)
    dmask = nc.dram_tensor("dmask", [P, 2 * P], FP8, kind="ExternalInput")
    slots_d = nc.dram_tensor("slots", [P, MT * NB], F32, kind="ExternalOutput")
    pos_d = nc.dram_tensor("pos", [P, MT], F32, kind="ExternalOutput")
    col_d = nc.dram_tensor("colsum", [3, CW], F32, kind="ExternalOutput")

    templates = {}

    with tile.TileContext(nc) as tc:
        with (
            tc.tile_pool(name="singles", bufs=1) as singles,
            tc.tile_pool(name="epool", bufs=3) as epool,
            tc.tile_pool(name="psum_g", bufs=3, space="PSUM") as psum_g,
            tc.tile_pool(name="psum_c", bufs=1, space="PSUM") as psum_c,
        ):
            zt_sb = [singles.tile([P, KT, CW], FP8, name=f"zt{j}")
                     for j in range(NB)]
            dm_sb = singles.tile([P, 2 * P], FP8)
            I128 = singles.tile([P, P], F32)
            ones_bf = singles.tile([P, 1], BF16)
            slots = singles.tile([P, MT * NB], F32)
            pos = singles.tile([P, MT], F32)
            junk_pos = singles.tile([P, P], F32)
            colsb = [singles.tile([1, CW], F32, name=f"colsb{b}")
                     for b in range(1, 4)]
            # wait-carrier scratches (one per engine, never read)
            scr_v = singles.tile([1, 1], F32)
            scr_a = singles.tile([1, 1], F32)
            scr_p = singles.tile([1, 1], F32)

            # --- wait-carrier templates (harmless one-off ops) ---
            c0 = nc.const_aps.tensor(0.0, (1, 1), F32)
            templates[mybir.EngineType.DVE] = nc.vector.memset(scr_v[:], 0).ins
            templates[mybir.EngineType.Activation] = nc.scalar.copy(
                scr_a[:], c0).ins
            templates[mybir.EngineType.Pool] = nc.gpsimd.memset(scr_p[:], 0).ins
            templates[mybir.EngineType.PE] = nc.tensor.ldweights(
                ones_bf[:]).ins

            make_identity(nc, I128[:, :])
            nc.vector.memset(ones_bf[:], 1.0)

            nc.gpsimd.dma_start(out=dm_sb[:], in_=dmask[:, :])
            for j in range(NB):
                nc.gpsimd.dma_start(
                    out=zt_sb[j][:, :, :],
                    in_=zt[j * P:(j + 1) * P, :])

            # cps[h] accumulates column sums over m for the current jc
            cps = [psum_c.tile([1, 512], F32, name=f"cps{h}")
                   for h in range(CW // 512)]

            def emit_main(jc, m):
                g = psum_g.tile([P, CW], F32, tag="g")
                dh = m // (512 // P)          # h-region containing the diag
                for dk in range(DKT):
                    lhsT = zt_sb[0][:, 2 * dk:2 * dk + 2, m * P:(m + 1) * P]
                    for h in range(CW // 512):
                        stop = (dk == DKT - 1) and not (jc == 0 and h == dh)
                        nc.tensor.matmul(
                            g[:, h * 512:(h + 1) * 512],
                            lhsT,
                            zt_sb[jc][:, 2 * dk:2 * dk + 2,
                                      h * 512:(h + 1) * 512],
                            start=(dk == 0), stop=stop,
                            perf_mode=DR,
                            skip_group_check=True)
                if jc == 0:
                    # self-diag -> -DMV^2 via diag(-DMV).T @ diag(+DMV)
                    off = m * P
                    nc.tensor.matmul(
                        g[:, off:off + P],
                        dm_sb[:, 0:P], dm_sb[:, P:2 * P],
                        start=False, stop=True,
                        skip_group_check=True)
                if jc == 4:
                    # positive pair: rotated column = row + 4096 (reads g,
                    # doesn't block the exp)
                    off = m * P
                    nc.vector.scalar_tensor_tensor(
                        out=junk_pos[:], in0=g[:, off:off + P], scalar=1.0,
                        in1=I128[:], op0=ALU.mult, op1=ALU.mult,
                        accum_out=pos[:, m:m + 1])
                e = epool.tile([P, CW], BF16, tag="e")
                nc.scalar.activation(
                    out=e[:], in_=g[:], func=ACTF.Exp, scale=INVT2,
                    accum_out=slots[:, m * NB + jc:m * NB + jc + 1])
                return e

            def emit_colsum(jc, m, e):
                # partition-axis reduction of exp tile, accumulated over m
                for h in range(CW // 512):
                    nc.tensor.matmul(
                        cps[h][0:1, :],
                        ones_bf[:],
                        e[:, h * 512:(h + 1) * 512],
                        start=(m == 0), stop=(m == MT - 1),
                        skip_group_check=True)

            def flush_colsum(jc):
                for h in range(CW // 512):
                    nc.vector.tensor_copy(
                        colsb[jc - 1][:, h * 512:(h + 1) * 512],
                        cps[h][0:1, :])
                nc.gpsimd.dma_start(
                    out=col_d[jc - 1:jc, :], in_=colsb[jc - 1][:])

            # colsum(jc, m) is emitted after main(jc, m+1) so the PE never
            # sits waiting for ACT's exp of the tile it just produced.
            pending = None        # (jc, m, e) colsum not yet emitted
            for jc in range(NB):
                for m in range(MT):
                    e = emit_main(jc, m)
                    if pending is not None:
                        pj, pm, pe_t = pending
                        emit_colsum(pj, pm, pe_t)
                        pending = None
                        if pm == MT - 1:
                            flush_colsum(pj)
                    if 1 <= jc <= 3:
                        pending = (jc, m, e)
            assert pending is None

            nc.gpsimd.dma_start(out=slots_d[:, :], in_=slots[:])
            nc.gpsimd.dma_start(out=pos_d[:, :], in_=pos[:])

    _split_multi_waits(nc, templates)
    return nc


def _prep_core_input(q8, c):
    """q8: [8192, 1024] fp8 (normalized*S). Returns the [640, 8192] fp8
    array for core c: rotated rows (own block first), first 5 blocks,
    k-subtile-major layout."""
    zr = np.roll(q8, -c * MY, axis=0)[:NB * MY]          # [5120, 1024]
    # chunk jc: [1024 cols][8 ks][128 p] -> [128 p][8 ks][1024 cols]
    a = zr.reshape(NB, CW, KT, P).transpose(0, 3, 2, 1)  # [5, 128, 8, 1024]
    return np.ascontiguousarray(a.reshape(NB * P, KT * CW))


def kernel(z_i: np.ndarray, z_j: np.ndarray) -> np.ndarray:
    global _NC_CACHE, LAST_RESULTS
    z = np.concatenate([np.asarray(z_i, dtype=np.float32),
                        np.asarray(z_j, dtype=np.float32)], axis=0)
    norm = np.maximum(np.sqrt((z.astype(np.float64) ** 2).sum(axis=1,
                                                              keepdims=True)),
                      1e-8)
    q8 = ((z / norm) * S).astype(ml_dtypes.float8_e4m3)

    dm = np.zeros((P, 2 * P), dtype=ml_dtypes.float8_e4m3)
    idx = np.arange(P)
    dm[idx, idx] = -DMV
    dm[idx, P + idx] = DMV

    in_maps = [{"zt": _prep_core_input(q8, c), "dmask": dm} for c in range(8)]

    if _NC_CACHE is None:
        _NC_CACHE = build()

    res = run_bass_kernel_spmd(
        _NC_CACHE, in_maps, core_ids=list(range(8)), trace=TRACE)
    LAST_RESULTS = res

    sumexp = np.zeros(R, np.float64)
    pos_g = np.zeros(R, np.float64)
    for c in range(8):
        slots = res.results[c]["slots"].astype(np.float64)   # [128, m*5+jc]
        rs = slots.reshape(P, MT, NB).sum(axis=2)            # [p, m]
        sumexp[c * MY:(c + 1) * MY] += rs.T.reshape(MY)      # row i = m*128+p
        posv = res.results[c]["pos"].astype(np.float64)      # [p, m]
        pos_g[c * MY:(c + 1) * MY] = posv.T.reshape(MY) * INVT2
        col = res.results[c]["colsum"].astype(np.float64)    # [3, 1024]
        for b in (1, 2, 3):
            gb = (c + b) % 8
            sumexp[gb * MY:(gb + 1) * MY] += col[b - 1]
    loss = np.mean(np.log(sumexp) - pos_g)
    return np.float32(loss)


# revision 16
# speedup vs baseline: 1.1562x; 1.1562x over previous
"""NT-Xent contrastive loss (SimCLR) on 8 Trainium2 NeuronCores.

Strategy (v3: host-norm + symmetry + fp8 DoubleRow, PE-only dep chains):
  - Host: z = concat(z_i, z_j) [8192, 1024], L2-normalize rows in f32,
    scale by S=16, quantize to fp8 e4m3. The cosine-sim matrix is then
    just G = q @ q.T (scaled by S^2), no on-device normalization.
  - Symmetry: sim is symmetric. In rotated coords (each core's 1024 rows
    at block 0), core c computes only column blocks 0..4 (5/8 of the
    matrix). Blocks 1-3 contribute BOTH row-side exp-sums (via ACT accum)
    and column-side exp-sums (ones-matmul partition reduction of the exp
    tiles, accumulated over m in PSUM); block 4 row-side only (its
    transpose is block 4 of the peer core); block 0 row-side with the
    self-diagonal masked. Every ordered (r,c) pair of the full 8192x8192
    matrix is covered exactly once across the fleet.
  - PE runs fp8e4 DoubleRow matmuls (2 k-subtiles of 128 per pass,
    0.5 cycles/row = 4x bf16 MAC throughput). Contraction 1024 = 4
    DoubleRow groups accumulated in PSUM.
  - Self-diag masked ON PE: an extra [128,128] fp8 matmul diag(-28) x
    diag(+28) = -784*I appended to the accumulation group (G_diag ~ +256,
    so masked logit ~ -529*INVT2 ~ -29.5 -> exp ~ 1.5e-13, negligible).
    This keeps every ACT exp dependent only on the PE semaphore (no DVE
    writes to PSUM), so Tile emits a single merged sem wait per exp and
    the multi-wait splitter leaves ACT alone.
  - Column-side reduction mostly off PE: DVE accumulates sum_m E_m in
    bf16; PE collapses the partition axis with two ones-matmuls per block
    (1.3us total; walrus here cannot codegen gpsimd partition_all_reduce).
  - Host combines: per-row sumexp = own row-side + 3 column-side chunks
    from neighbor cores; loss = mean(log(sumexp) - pos/(S^2*T)).

This container's walrus build only accepts ONE semaphore wait per
instruction (and none on CTRL-encoded ones like Drain), while Tile freely
emits several. Two workarounds below: the TileContext epilogue drain's waits
are re-emitted on DVE memsets, and a post-pass splits any multi-wait
instruction by inserting single-wait no-op "carrier" clones (per-engine
templates) just before it on the same engine stream. The PE carrier is a
[128,1] LDWEIGHTS (harmless: every real matmul loads its own weights).
"""

import copy

import numpy as np
import ml_dtypes


def _install_tile_drain_patch():
    import concourse.tile as tile
    from concourse import mybir
    from concourse.vector_clock import ScopedClock

    if getattr(tile.TileContext, "_drain_patch_installed", False):
        return

    def _drain_and_barrier(self, tick_clock, wait_clock):
        nc = self.nc
        drain_inst = nc.sync.drain()
        wait_clock.add_sem_waits(
            drain_inst.ins, ScopedClock({None: tick_clock.global_clock})
        )
        waits = list(drain_inst.ins.sync_info.on_wait)
        drain_inst.ins.sync_info.on_wait.clear()

        if waits:
            scr = nc.const_aps.tensor(0.0, (1, 1), mybir.dt.float32)
            for w in waits:
                ms = nc.vector.memset(scr, 0)
                if ms.ins.sync_info is None:
                    ms.ins.sync_info = mybir.SyncInfo(on_wait=[], on_update=[])
                ms.ins.sync_info.on_wait.append(w)

        nc.all_engine_barrier()
        assert self.sems is not None
        popped = nc._tile_sem_poison_stack.pop()
        assert popped is self._sem_poison
        nc.clear_and_free_semaphores(list(self.sems.allocated().values()))
        nc.all_engine_barrier()

    tile.TileContext._drain_and_barrier = _drain_and_barrier
    tile.TileContext._drain_patch_installed = True


_install_tile_drain_patch()

import concourse.bass as bass
import concourse.tile as tile
from concourse import mybir
from concourse.bass_utils import run_bass_kernel_spmd
from concourse.masks import make_identity

P = 128
D = 1024
R = 8192          # 2N rows
MY = 1024         # rows per core (= block size)
NB = 5            # column blocks computed per core (symmetry: 0..4)
KT = 8            # 128-deep k-subtiles in D
DKT = 4           # DoubleRow groups (256-deep each)
MT = 8            # m-tiles per core
CW = 1024         # column chunk width (= one block)
TEMP = 0.07
S = 16.0          # fp8 pre-scale; G = S^2 * sim
DMV = 28.0        # diag-mask matmul operand: adds -DMV^2 to self-sim
INVT2 = float(1.0 / (S * S * TEMP))
FP8 = mybir.dt.float8e4
BF16 = mybir.dt.bfloat16
F32 = mybir.dt.float32
ALU = mybir.AluOpType
ACTF = mybir.ActivationFunctionType
DR = mybir.MatmulPerfMode.DoubleRow

TRACE = False          # set True externally (test harness) for NTFF profiling
LAST_RESULTS = None    # BassKernelResults of the last run (for the harness)

_NC_CACHE = None


def _split_multi_waits(nc, templates):
    """Rewrite any instruction carrying >1 sem waits: keep the last wait,
    move each extra onto a fresh single-wait clone of the same-engine no-op
    template inserted immediately before it (engine streams are in-order)."""
    n = 0
    for f in nc.m.functions:
        for bb in f.blocks:
            newlist = []
            for ins in bb.instructions:
                si = getattr(ins, "sync_info", None)
                if si is not None and si.on_wait and len(si.on_wait) > 1:
                    extras = list(si.on_wait[:-1])
                    keep = list(si.on_wait[-1:])
                    tmpl = templates.get(ins.engine)
                    assert tmpl is not None, (
                        f"no wait-carrier template for engine {ins.engine} "
                        f"({type(ins).__name__} {ins.name})"
                    )
                    for w in extras:
                        c = copy.deepcopy(tmpl)
                        c.name = f"wcarrier-{n}"
                        n += 1
                        c.sync_info = mybir.SyncInfo(on_wait=[w], on_update=[])
                        newlist.append(c)
                    del si.on_wait[:]
                    si.on_wait.extend(keep)
                newlist.append(ins)
            bb.instructions[:] = newlist
    return n


def build():
    nc = bass.Bass()
    # [jc][p][ks][col]: element (p, ks, col) of chunk jc = q_rot[jc*CW+col,
    # ks*128+p]; flattened to rows jc*128+p, free ks*CW+col.
    zt = nc.dram_tensor("zt", [NB * P, KT * CW], FP8, kind="ExternalInput")
    # [:, :128] = diag(-DMV), [:, 128:] = diag(+DMV)
    dmask = nc.dram_tensor("dmask", [P, 2 * P], FP8, kind="ExternalInput")
    slots_d = nc.dram_tensor("slots", [P, MT * NB], F32, kind="ExternalOutput")
    pos_d = nc.dram_tensor("pos", [P, MT], F32, kind="ExternalOutput")
    col_d = nc.dram_tensor("colsum", [3, CW], F32, kind="ExternalOutput")

    templates = {}

    with tile.TileContext(nc) as tc:
        with (
            tc.tile_pool(name="singles", bufs=1) as singles,
            tc.tile_pool(name="epool", bufs=3) as epool,
            tc.tile_pool(name="psum_g", bufs=3, space="PSUM") as psum_g,
            tc.tile_pool(name="psum_c", bufs=1, space="PSUM") as psum_c,
        ):
            zt_sb = [singles.tile([P, KT, CW], FP8, name=f"zt{j}")
                     for j in range(NB)]
            dm_sb = singles.tile([P, 2 * P], FP8)
            I128 = singles.tile([P, P], F32)
            ones_bf = singles.tile([P, 1], BF16)
            slots = singles.tile([P, MT * NB], F32)
            pos = singles.tile([P, MT], F32)
            junk_pos = singles.tile([P, P], F32)
            acc = [singles.tile([P, CW], BF16, name=f"acc{b}")
                   for b in range(1, 4)]
            colsb = [singles.tile([1, CW], F32, name=f"colsb{b}")
                     for b in range(1, 4)]
            # wait-carrier scratches (one per engine, never read)
            scr_v = singles.tile([1, 1], F32)
            scr_a = singles.tile([1, 1], F32)
            scr_p = singles.tile([1, 1], F32)

            # --- wait-carrier templates (harmless one-off ops) ---
            c0 = nc.const_aps.tensor(0.0, (1, 1), F32)
            templates[mybir.EngineType.DVE] = nc.vector.memset(scr_v[:], 0).ins
            templates[mybir.EngineType.Activation] = nc.scalar.copy(
                scr_a[:], c0).ins
            templates[mybir.EngineType.Pool] = nc.gpsimd.memset(scr_p[:], 0).ins
            templates[mybir.EngineType.PE] = nc.tensor.ldweights(
                ones_bf[:]).ins

            make_identity(nc, I128[:, :])
            nc.vector.memset(ones_bf[:], 1.0)

            # chunk 0 arrives in dk-granular slices so the first matmul
            # group only waits for 256 KB, not the full megabyte
            for dk in range(DKT):
                nc.gpsimd.dma_start(
                    out=zt_sb[0][:, 2 * dk:2 * dk + 2, :],
                    in_=zt[0:P, 2 * dk * CW:(2 * dk + 2) * CW])
            nc.gpsimd.dma_start(out=dm_sb[:], in_=dmask[:, :])
            for j in range(1, NB):
                nc.gpsimd.dma_start(
                    out=zt_sb[j][:, :, :],
                    in_=zt[j * P:(j + 1) * P, :])

            def emit_main(jc, m):
                g = psum_g.tile([P, CW], F32, tag="g")
                dh = m // (512 // P)          # h-region containing the diag
                for dk in range(DKT):
                    lhsT = zt_sb[0][:, 2 * dk:2 * dk + 2, m * P:(m + 1) * P]
                    for h in range(CW // 512):
                        stop = (dk == DKT - 1) and not (jc == 0 and h == dh)
                        nc.tensor.matmul(
                            g[:, h * 512:(h + 1) * 512],
                            lhsT,
                            zt_sb[jc][:, 2 * dk:2 * dk + 2,
                                      h * 512:(h + 1) * 512],
                            start=(dk == 0), stop=stop,
                            perf_mode=DR,
                            skip_group_check=True)
                if jc == 0:
                    # self-diag -> -DMV^2 via diag(-DMV).T @ diag(+DMV)
                    off = m * P
                    nc.tensor.matmul(
                        g[:, off:off + P],
                        dm_sb[:, 0:P], dm_sb[:, P:2 * P],
                        start=False, stop=True,
                        skip_group_check=True)
                if jc == 4:
                    # positive pair: rotated column = row + 4096 (reads g,
                    # doesn't block the exp)
                    off = m * P
                    nc.vector.scalar_tensor_tensor(
                        out=junk_pos[:], in0=g[:, off:off + P], scalar=1.0,
                        in1=I128[:], op0=ALU.mult, op1=ALU.mult,
                        accum_out=pos[:, m:m + 1])
                e = epool.tile([P, CW], BF16, tag="e")
                nc.scalar.activation(
                    out=e[:], in_=g[:], func=ACTF.Exp, scale=INVT2,
                    accum_out=slots[:, m * NB + jc:m * NB + jc + 1])
                if 1 <= jc <= 3:
                    # column-side partial: acc_b += E_m (bf16, DVE)
                    if m == 0:
                        nc.vector.tensor_copy(acc[jc - 1][:], e[:])
                    else:
                        nc.vector.tensor_tensor(
                            out=acc[jc - 1][:], in0=acc[jc - 1][:],
                            in1=e[:], op=ALU.add)

            cps = [psum_c.tile([1, 512], F32, name=f"cps{h}")
                   for h in range(CW // 512)]

            for jc in range(NB):
                for m in range(MT):
                    emit_main(jc, m)
                if 1 <= jc <= 3:
                    # collapse partition axis: ones-matmul per 512-col half
                    for h in range(CW // 512):
                        nc.tensor.matmul(
                            cps[h][0:1, :],
                            ones_bf[:],
                            acc[jc - 1][:, h * 512:(h + 1) * 512],
                            start=True, stop=True,
                            skip_group_check=True)
                        nc.vector.tensor_copy(
                            colsb[jc - 1][:, h * 512:(h + 1) * 512],
                            cps[h][0:1, :])
                    nc.gpsimd.dma_start(
                        out=col_d[jc - 1:jc, :], in_=colsb[jc - 1][:])

            nc.gpsimd.dma_start(out=slots_d[:, :], in_=slots[:])
            nc.gpsimd.dma_start(out=pos_d[:, :], in_=pos[:])

    _split_multi_waits(nc, templates)
    return nc


def _prep_core_input(q8, c):
    """q8: [8192, 1024] fp8 (normalized*S). Returns the [640, 8192] fp8
    array for core c: rotated rows (own block first), first 5 blocks,
    k-subtile-major layout."""
    zr = np.roll(q8, -c * MY, axis=0)[:NB * MY]          # [5120, 1024]
    # chunk jc: [1024 cols][8 ks][128 p] -> [128 p][8 ks][1024 cols]
    a = zr.reshape(NB, CW, KT, P).transpose(0, 3, 2, 1)  # [5, 128, 8, 1024]
    return np.ascontiguousarray(a.reshape(NB * P, KT * CW))


def kernel(z_i: np.ndarray, z_j: np.ndarray) -> np.ndarray:
    global _NC_CACHE, LAST_RESULTS
    z = np.concatenate([np.asarray(z_i, dtype=np.float32),
                        np.asarray(z_j, dtype=np.float32)], axis=0)
    norm = np.maximum(np.sqrt((z.astype(np.float64) ** 2).sum(axis=1,
                                                              keepdims=True)),
                      1e-8)
    q8 = ((z / norm) * S).astype(ml_dtypes.float8_e4m3)

    dm = np.zeros((P, 2 * P), dtype=ml_dtypes.float8_e4m3)
    idx = np.arange(P)
    dm[idx, idx] = -DMV
    dm[idx, P + idx] = DMV

    in_maps = [{"zt": _prep_core_input(q8, c), "dmask": dm} for c in range(8)]

    if _NC_CACHE is None:
        _NC_CACHE = build()

    res = run_bass_kernel_spmd(
        _NC_CACHE, in_maps, core_ids=list(range(8)), trace=TRACE)
    LAST_RESULTS = res

    sumexp = np.zeros(R, np.float64)
    pos_g = np.zeros(R, np.float64)
    for c in range(8):
        slots = res.results[c]["slots"].astype(np.float64)   # [128, m*5+jc]
        rs = slots.reshape(P, MT, NB).sum(axis=2)            # [p, m]
        sumexp[c * MY:(c + 1) * MY] += rs.T.reshape(MY)      # row i = m*128+p
        posv = res.results[c]["pos"].astype(np.float64)      # [p, m]
        pos_g[c * MY:(c + 1) * MY] = posv.T.reshape(MY) * INVT2
        col = res.results[c]["colsum"].astype(np.float64)    # [3, 1024]
        for b in (1, 2, 3):
            gb = (c + b) % 8
            sumexp[gb * MY:(gb + 1) * MY] += col[b - 1]
    loss = np.mean(np.log(sumexp) - pos_g)
    return np.float32(loss)


# revision 18
# speedup vs baseline: 1.1659x; 1.0083x over previous
"""NT-Xent contrastive loss (SimCLR) on 8 Trainium2 NeuronCores.

Strategy (v3: host-norm + symmetry + fp8 DoubleRow, PE-only dep chains):
  - Host: z = concat(z_i, z_j) [8192, 1024], L2-normalize rows in f32,
    scale by S=16, quantize to fp8 e4m3. The cosine-sim matrix is then
    just G = q @ q.T (scaled by S^2), no on-device normalization.
  - Symmetry: sim is symmetric. In rotated coords (each core's 1024 rows
    at block 0), core c computes only column blocks 0..4 (5/8 of the
    matrix). Blocks 1-3 contribute BOTH row-side exp-sums (via ACT accum)
    and column-side exp-sums (ones-matmul partition reduction of the exp
    tiles, accumulated over m in PSUM); block 4 row-side only (its
    transpose is block 4 of the peer core); block 0 row-side with the
    self-diagonal masked. Every ordered (r,c) pair of the full 8192x8192
    matrix is covered exactly once across the fleet.
  - PE runs fp8e4 DoubleRow matmuls (2 k-subtiles of 128 per pass,
    0.5 cycles/row = 4x bf16 MAC throughput). Contraction 1024 = 4
    DoubleRow groups accumulated in PSUM.
  - Self-diag masked ON PE: an extra [128,128] fp8 matmul diag(-28) x
    diag(+28) = -784*I appended to the accumulation group (G_diag ~ +256,
    so masked logit ~ -529*INVT2 ~ -29.5 -> exp ~ 1.5e-13, negligible).
    This keeps every ACT exp dependent only on the PE semaphore (no DVE
    writes to PSUM), so Tile emits a single merged sem wait per exp and
    the multi-wait splitter leaves ACT alone.
  - Column-side reduction mostly off PE: DVE accumulates sum_m E_m in
    bf16; PE collapses the partition axis with two ones-matmuls per block
    (1.3us total; walrus here cannot codegen gpsimd partition_all_reduce).
  - Host combines: per-row sumexp = own row-side + 3 column-side chunks
    from neighbor cores; loss = mean(log(sumexp) - pos/(S^2*T)).

This container's walrus build only accepts ONE semaphore wait per
instruction (and none on CTRL-encoded ones like Drain), while Tile freely
emits several. Two workarounds below: the TileContext epilogue drain's waits
are re-emitted on DVE memsets, and a post-pass splits any multi-wait
instruction by inserting single-wait no-op "carrier" clones (per-engine
templates) just before it on the same engine stream. The PE carrier is a
[128,1] LDWEIGHTS (harmless: every real matmul loads its own weights).
"""

import copy

import numpy as np
import ml_dtypes


def _install_tile_drain_patch():
    import concourse.tile as tile
    from concourse import mybir
    from concourse.vector_clock import ScopedClock

    if getattr(tile.TileContext, "_drain_patch_installed", False):
        return

    def _drain_and_barrier(self, tick_clock, wait_clock):
        nc = self.nc
        drain_inst = nc.sync.drain()
        wait_clock.add_sem_waits(
            drain_inst.ins, ScopedClock({None: tick_clock.global_clock})
        )
        waits = list(drain_inst.ins.sync_info.on_wait)
        drain_inst.ins.sync_info.on_wait.clear()

        if waits:
            scr = nc.const_aps.tensor(0.0, (1, 1), mybir.dt.float32)
            for w in waits:
                ms = nc.vector.memset(scr, 0)
                if ms.ins.sync_info is None:
                    ms.ins.sync_info = mybir.SyncInfo(on_wait=[], on_update=[])
                ms.ins.sync_info.on_wait.append(w)

        nc.all_engine_barrier()
        assert self.sems is not None
        popped = nc._tile_sem_poison_stack.pop()
        assert popped is self._sem_poison
        nc.clear_and_free_semaphores(list(self.sems.allocated().values()))
        nc.all_engine_barrier()

    tile.TileContext._drain_and_barrier = _drain_and_barrier
    tile.TileContext._drain_patch_installed = True


_install_tile_drain_patch()

import concourse.bass as bass
import concourse.tile as tile
from concourse import mybir
from concourse.bass_utils import run_bass_kernel_spmd
from concourse.masks import make_identity

P = 128
D = 1024
R = 8192          # 2N rows
MY = 1024         # rows per core (= block size)
NB = 5            # column blocks computed per core (symmetry: 0..4)
KT = 8            # 128-deep k-subtiles in D
DKT = 4           # DoubleRow groups (256-deep each)
MT = 8            # m-tiles per core
CW = 1024         # column chunk width (= one block)
TEMP = 0.07
S = 16.0          # fp8 pre-scale; G = S^2 * sim
DMV = 28.0        # diag-mask matmul operand: adds -DMV^2 to self-sim
INVT2 = float(1.0 / (S * S * TEMP))
FP8 = mybir.dt.float8e4
BF16 = mybir.dt.bfloat16
F32 = mybir.dt.float32
ALU = mybir.AluOpType
ACTF = mybir.ActivationFunctionType
DR = mybir.MatmulPerfMode.DoubleRow

TRACE = False          # set True externally (test harness) for NTFF profiling
LAST_RESULTS = None    # BassKernelResults of the last run (for the harness)

_NC_CACHE = None


_COMPUTE_INSTS = {
    "InstMatmult", "InstLdweights", "InstActivation", "InstTensorTensor",
    "InstTensorScalarPtr", "InstTensorCopy", "InstMemset", "InstTensorReduce",
    "InstTensorScalarAffineSelect",
}


def _elide_self_waits(nc):
    """Drop semaphore waits that are trivially satisfied by same-engine
    program order: a wait on a sem that is only ever incremented by compute
    instructions of the waiting instruction's own engine, with a target
    value already reached by the updates of instructions earlier in that
    (in-order) engine stream."""
    updaters = {}       # sem id -> set of (engine, inst type)
    for f in nc.m.functions:
        for bb in f.blocks:
            for ins in bb.instructions:
                si = getattr(ins, "sync_info", None)
                if si is None:
                    continue
                for u in si.on_update:
                    if u.update_mode != "sem-inc":
                        updaters.setdefault(u.id, set()).add(("!", "!"))
                        continue
                    updaters.setdefault(u.id, set()).add(
                        (ins.engine, type(ins).__name__))
    self_sems = {}      # sem id -> engine (safe to elide for that engine)
    for sid, kinds in updaters.items():
        engines = {e for e, _ in kinds}
        types = {t for _, t in kinds}
        if len(engines) == 1 and types <= _COMPUTE_INSTS:
            self_sems[sid] = next(iter(engines))

    counts = {}         # (engine, sem id) -> completed increments so far
    n = 0
    for f in nc.m.functions:
        for bb in f.blocks:
            for ins in bb.instructions:
                si = getattr(ins, "sync_info", None)
                if si is None:
                    continue
                if si.on_wait:
                    keep = []
                    for w in si.on_wait:
                        eng = self_sems.get(w.id)
                        if (eng == ins.engine
                                and w.wait_mode == "sem-ge-imm"
                                and w.wait_value <= counts.get(
                                    (eng, w.id), 0)):
                            n += 1
                        else:
                            keep.append(w)
                    if len(keep) != len(si.on_wait):
                        del si.on_wait[:]
                        si.on_wait.extend(keep)
                for u in si.on_update:
                    if u.id in self_sems and self_sems[u.id] == ins.engine:
                        counts[(ins.engine, u.id)] = (
                            counts.get((ins.engine, u.id), 0) + u.update_value)
    return n


def _split_multi_waits(nc, templates):
    """Rewrite any instruction carrying >1 sem waits: keep the last wait,
    move each extra onto a fresh single-wait clone of the same-engine no-op
    template inserted immediately before it (engine streams are in-order)."""
    n = 0
    for f in nc.m.functions:
        for bb in f.blocks:
            newlist = []
            for ins in bb.instructions:
                si = getattr(ins, "sync_info", None)
                if si is not None and si.on_wait and len(si.on_wait) > 1:
                    extras = list(si.on_wait[:-1])
                    keep = list(si.on_wait[-1:])
                    tmpl = templates.get(ins.engine)
                    assert tmpl is not None, (
                        f"no wait-carrier template for engine {ins.engine} "
                        f"({type(ins).__name__} {ins.name})"
                    )
                    for w in extras:
                        c = copy.deepcopy(tmpl)
                        c.name = f"wcarrier-{n}"
                        n += 1
                        c.sync_info = mybir.SyncInfo(on_wait=[w], on_update=[])
                        newlist.append(c)
                    del si.on_wait[:]
                    si.on_wait.extend(keep)
                newlist.append(ins)
            bb.instructions[:] = newlist
    return n


def build():
    nc = bass.Bass()
    # [jc][p][ks][col]: element (p, ks, col) of chunk jc = q_rot[jc*CW+col,
    # ks*128+p]; flattened to rows jc*128+p, free ks*CW+col.
    zt = nc.dram_tensor("zt", [NB * P, KT * CW], FP8, kind="ExternalInput")
    # [:, :128] = diag(-DMV), [:, 128:] = diag(+DMV)
    dmask = nc.dram_tensor("dmask", [P, 2 * P], FP8, kind="ExternalInput")
    slots_d = nc.dram_tensor("slots", [P, MT * NB], F32, kind="ExternalOutput")
    pos_d = nc.dram_tensor("pos", [P, MT], F32, kind="ExternalOutput")
    col_d = nc.dram_tensor("colsum", [3, CW], F32, kind="ExternalOutput")

    templates = {}

    with tile.TileContext(nc) as tc:
        with (
            tc.tile_pool(name="singles", bufs=1) as singles,
            tc.tile_pool(name="epool", bufs=3) as epool,
            tc.tile_pool(name="psum_g", bufs=3, space="PSUM") as psum_g,
            tc.tile_pool(name="psum_c", bufs=1, space="PSUM") as psum_c,
        ):
            zt_sb = [singles.tile([P, KT, CW], FP8, name=f"zt{j}")
                     for j in range(NB)]
            dm_sb = singles.tile([P, 2 * P], FP8)
            I128 = singles.tile([P, P], F32)
            ones_bf = singles.tile([P, 1], BF16)
            slots = singles.tile([P, MT * NB], F32)
            pos = singles.tile([P, MT], F32)
            junk_pos = singles.tile([P, P], F32)
            acc = [singles.tile([P, CW], BF16, name=f"acc{b}")
                   for b in range(1, 4)]
            colsb = [singles.tile([1, CW], F32, name=f"colsb{b}")
                     for b in range(1, 4)]
            # wait-carrier scratches (one per engine, never read)
            scr_v = singles.tile([1, 1], F32)
            scr_a = singles.tile([1, 1], F32)
            scr_p = singles.tile([1, 1], F32)

            # --- wait-carrier templates (harmless one-off ops) ---
            c0 = nc.const_aps.tensor(0.0, (1, 1), F32)
            templates[mybir.EngineType.DVE] = nc.vector.memset(scr_v[:], 0).ins
            templates[mybir.EngineType.Activation] = nc.scalar.copy(
                scr_a[:], c0).ins
            templates[mybir.EngineType.Pool] = nc.gpsimd.memset(scr_p[:], 0).ins
            templates[mybir.EngineType.PE] = nc.tensor.ldweights(
                ones_bf[:]).ins

            make_identity(nc, I128[:, :])
            nc.vector.memset(ones_bf[:], 1.0)

            # chunk 0 arrives in dk-granular slices so the first matmul
            # group only waits for 256 KB, not the full megabyte
            for dk in range(DKT):
                nc.gpsimd.dma_start(
                    out=zt_sb[0][:, 2 * dk:2 * dk + 2, :],
                    in_=zt[0:P, 2 * dk * CW:(2 * dk + 2) * CW])
            nc.gpsimd.dma_start(out=dm_sb[:], in_=dmask[:, :])
            for j in range(1, NB):
                nc.gpsimd.dma_start(
                    out=zt_sb[j][:, :, :],
                    in_=zt[j * P:(j + 1) * P, :])

            def emit_main(jc, m):
                g = psum_g.tile([P, CW], F32, tag="g")
                dh = m // (512 // P)          # h-region containing the diag
                for dk in range(DKT):
                    lhsT = zt_sb[0][:, 2 * dk:2 * dk + 2, m * P:(m + 1) * P]
                    for h in range(CW // 512):
                        stop = (dk == DKT - 1) and not (jc == 0 and h == dh)
                        nc.tensor.matmul(
                            g[:, h * 512:(h + 1) * 512],
                            lhsT,
                            zt_sb[jc][:, 2 * dk:2 * dk + 2,
                                      h * 512:(h + 1) * 512],
                            start=(dk == 0), stop=stop,
                            perf_mode=DR,
                            skip_group_check=True)
                if jc == 0:
                    # self-diag -> -DMV^2 via diag(-DMV).T @ diag(+DMV)
                    off = m * P
                    nc.tensor.matmul(
                        g[:, off:off + P],
                        dm_sb[:, 0:P], dm_sb[:, P:2 * P],
                        start=False, stop=True,
                        skip_group_check=True)
                if jc == 4:
                    # positive pair: rotated column = row + 4096 (reads g,
                    # doesn't block the exp)
                    off = m * P
                    nc.vector.scalar_tensor_tensor(
                        out=junk_pos[:], in0=g[:, off:off + P], scalar=1.0,
                        in1=I128[:], op0=ALU.mult, op1=ALU.mult,
                        accum_out=pos[:, m:m + 1])
                e = epool.tile([P, CW], BF16, tag="e")
                nc.scalar.activation(
                    out=e[:], in_=g[:], func=ACTF.Exp, scale=INVT2,
                    accum_out=slots[:, m * NB + jc:m * NB + jc + 1])
                if 1 <= jc <= 3:
                    # column-side partial: acc_b += E_m (bf16, DVE)
                    if m == 0:
                        nc.vector.tensor_copy(acc[jc - 1][:], e[:])
                    else:
                        nc.vector.tensor_tensor(
                            out=acc[jc - 1][:], in0=acc[jc - 1][:],
                            in1=e[:], op=ALU.add)

            cps = [psum_c.tile([1, 512], F32, name=f"cps{h}")
                   for h in range(CW // 512)]

            for jc in range(NB):
                for m in range(MT):
                    emit_main(jc, m)
                if 1 <= jc <= 3:
                    # collapse partition axis: ones-matmul per 512-col half
                    for h in range(CW // 512):
                        nc.tensor.matmul(
                            cps[h][0:1, :],
                            ones_bf[:],
                            acc[jc - 1][:, h * 512:(h + 1) * 512],
                            start=True, stop=True,
                            skip_group_check=True)
                        nc.vector.tensor_copy(
                            colsb[jc - 1][:, h * 512:(h + 1) * 512],
                            cps[h][0:1, :])
                    nc.gpsimd.dma_start(
                        out=col_d[jc - 1:jc, :], in_=colsb[jc - 1][:])

            nc.gpsimd.dma_start(out=slots_d[:, :], in_=slots[:])
            nc.gpsimd.dma_start(out=pos_d[:, :], in_=pos[:])

    _elide_self_waits(nc)
    _split_multi_waits(nc, templates)
    return nc


def _prep_core_input(q8, c):
    """q8: [8192, 1024] fp8 (normalized*S). Returns the [640, 8192] fp8
    array for core c: rotated rows (own block first), first 5 blocks,
    k-subtile-major layout."""
    zr = np.roll(q8, -c * MY, axis=0)[:NB * MY]          # [5120, 1024]
    # chunk jc: [1024 cols][8 ks][128 p] -> [128 p][8 ks][1024 cols]
    a = zr.reshape(NB, CW, KT, P).transpose(0, 3, 2, 1)  # [5, 128, 8, 1024]
    return np.ascontiguousarray(a.reshape(NB * P, KT * CW))


def kernel(z_i: np.ndarray, z_j: np.ndarray) -> np.ndarray:
    global _NC_CACHE, LAST_RESULTS
    z = np.concatenate([np.asarray(z_i, dtype=np.float32),
                        np.asarray(z_j, dtype=np.float32)], axis=0)
    norm = np.maximum(np.sqrt((z.astype(np.float64) ** 2).sum(axis=1,
                                                              keepdims=True)),
                      1e-8)
    q8 = ((z / norm) * S).astype(ml_dtypes.float8_e4m3)

    dm = np.zeros((P, 2 * P), dtype=ml_dtypes.float8_e4m3)
    idx = np.arange(P)
    dm[idx, idx] = -DMV
    dm[idx, P + idx] = DMV

    in_maps = [{"zt": _prep_core_input(q8, c), "dmask": dm} for c in range(8)]

    if _NC_CACHE is None:
        _NC_CACHE = build()

    res = run_bass_kernel_spmd(
        _NC_CACHE, in_maps, core_ids=list(range(8)), trace=TRACE)
    LAST_RESULTS = res

    sumexp = np.zeros(R, np.float64)
    pos_g = np.zeros(R, np.float64)
    for c in range(8):
        slots = res.results[c]["slots"].astype(np.float64)   # [128, m*5+jc]
        rs = slots.reshape(P, MT, NB).sum(axis=2)            # [p, m]
        sumexp[c * MY:(c + 1) * MY] += rs.T.reshape(MY)      # row i = m*128+p
        posv = res.results[c]["pos"].astype(np.float64)      # [p, m]
        pos_g[c * MY:(c + 1) * MY] = posv.T.reshape(MY) * INVT2
        col = res.results[c]["colsum"].astype(np.float64)    # [3, 1024]
        for b in (1, 2, 3):
            gb = (c + b) % 8
            sumexp[gb * MY:(gb + 1) * MY] += col[b - 1]
    loss = np.mean(np.log(sumexp) - pos_g)
    return np.float32(loss)
